# revision 5
# baseline (speedup 1.0000x reference)
"""AFeFET quantized linear layer on 8 TRN2 NeuronCores.

Reference computation:
  qv   = snap(4.5*(1 + w*a)) to nearest of {3.5,4.0,4.5,5.0,5.5}
  qw   = (qv/4.5 - 1)/a * exp(-1e-3) * (1 - clip(wc/1e8*0.1, 0, 0.5))
  y    = x @ qw.T          x:[8,2048,4096] f32, w:[4096,4096] f32, wc int64

Sharding: batch 2-way x out_features 4-way (8 cores).  Each core:
  xlin [8192,4096] f32r (tile-linearized transpose of its 4 batches)
  wT   [4096,1024] f32, wcT [4096,1024] int32, alpha [1,1] f32
  y    [8192,1024] f32

Device does the whole quantization chain (DVE+ACT) and the GEMM in
float32r (fp32 storage, ~1e-4 matmul precision, 4x the fp32 rate).
Host does only dtype-preserving layout prep (transpose/reshape) and the
lossless int64->int32 narrowing (values < 2^31).
"""
import sys
sys.path.insert(0, "/opt/trn_rl_repo")
import math
import numpy as np

import concourse.bass as bass
import concourse.mybir as mybir
import concourse.tile as tile
from concourse import bacc
from concourse.bass_utils import run_bass_kernel_spmd

P = 128
N_CORES = 8

# full-problem shape
B, S, IN_F, OUT_F = 8, 2048, 4096, 4096
BATCH_WAYS, OUT_WAYS = 2, 4
TOK = (B // BATCH_WAYS) * S          # 8192 tokens per core
O = OUT_F // OUT_WAYS                # 1024 out_features per core

C_DECAY = float(np.exp(np.float64(-0.001)) / 4.5)   # 0.22200011107408333


def build(tok=TOK, kin=IN_F, o=O, g_width=512):
    """Per-core SPMD graph. tok/kin multiples of 128, o multiple of g_width."""
    ksub = kin // P
    ntok = tok // P
    ngrp = o // g_width

    nc = bacc.Bacc("TRN2", target_bir_lowering=False, debug=False)
    xlin = nc.dram_tensor("xlin", [tok, kin], mybir.dt.float32r, kind="ExternalInput")
    wt = nc.dram_tensor("wt", [kin, o], mybir.dt.float32, kind="ExternalInput")
    wct = nc.dram_tensor("wct", [kin, o], mybir.dt.int32, kind="ExternalInput")
    alpha = nc.dram_tensor("alpha", [1, 1], mybir.dt.float32, kind="ExternalInput")
    y = nc.dram_tensor("y", [tok, o], mybir.dt.float32, kind="ExternalOutput")

    # xlin row t*P+p, col ks*P+c holds x.T[ks*P+p, t*P+c]: tile t DMAs as one
    # contiguous [P, kin] block straight into SBUF layout.
    xr = xlin.ap().rearrange("(t p) c -> t p c", p=P)

    with tile.TileContext(nc) as tc:
        with (
            tc.tile_pool(name="const", bufs=1) as constp,
            tc.tile_pool(name="wstage", bufs=2) as wstage,
            tc.tile_pool(name="qpool", bufs=1) as qpool,
            tc.tile_pool(name="tmp", bufs=1) as tmpp,
            tc.tile_pool(name="xpool", bufs=4) as xpool,
            tc.tile_pool(name="ypool", bufs=4) as ypool,
            tc.tile_pool(name="ps", bufs=4, space="PSUM") as ps,
        ):
            # ---- alpha-derived per-partition scalars (one padded tile) ----
            cs = constp.tile([P, 8], mybir.dt.float32)
            a_sb, s9a, rec, k1, kneg = (cs[:, i:i + 1] for i in range(5))
            alpha_bcast = bass.AP(tensor=alpha, offset=0, ap=[[0, P], [1, 1]])
            nc.gpsimd.dma_start(out=a_sb, in_=alpha_bcast)
            nc.vector.tensor_scalar_mul(s9a, a_sb, 9.0)
            nc.vector.reciprocal(rec, a_sb)
            nc.vector.tensor_scalar_mul(k1, rec, 0.5 * C_DECAY)
            nc.vector.tensor_scalar_mul(kneg, rec, -C_DECAY)

            # ---- quantization chain -> resident qwT [P, ksub, o] f32r ----
            qw = qpool.tile([P, ksub, o], mybir.dt.float32r)
            wtr = wt.ap().rearrange("(ks p) o -> ks p o", p=P)
            wctr = wct.ap().rearrange("(ks p) o -> ks p o", p=P)
            for ks in range(ksub):
                w_s = wstage.tile([P, o], mybir.dt.float32, name="w_s", tag="w_s")
                nc.sync.dma_start(w_s[:], wtr[ks])
                wc_s = wstage.tile([P, o], mybir.dt.int32, name="wc_s", tag="wc_s")
                nc.sync.dma_start(wc_s[:], wctr[ks])

                u = tmpp.tile([P, o], mybir.dt.int32, name="u", tag="u")
                nc.vector.tensor_scalar(u[:], w_s[:], s9a[:], 2.0,
                                        op0=mybir.AluOpType.mult,
                                        op1=mybir.AluOpType.add)
                u2 = tmpp.tile([P, o], mybir.dt.int32, name="u2", tag="u2")
                nc.vector.tensor_scalar(u2[:], u[:], 4, 0,
                                        op0=mybir.AluOpType.min,
                                        op1=mybir.AluOpType.max)
                q_f = tmpp.tile([P, o], mybir.dt.float32, name="q_f", tag="q_f")
                nc.scalar.activation(q_f[:], u2[:],
                                     mybir.ActivationFunctionType.Identity,
                                     bias=kneg[:], scale=k1[:])
                m_f = tmpp.tile([P, o], mybir.dt.float32, name="m_f", tag="m_f")
                nc.scalar.activation(m_f[:], wc_s[:],
                                     mybir.ActivationFunctionType.Identity,
                                     bias=1.0, scale=-1e-9)
                nc.vector.tensor_tensor(out=qw[:, ks], in0=q_f[:], in1=m_f[:],
                                        op=mybir.AluOpType.mult)

            # ---- GEMM: y[t*P:(t+1)*P, g*512:(g+1)*512] ----
            khalf = max(1, ksub // 2)
            for t in range(ntok):
                xh = []
                for h in range((ksub + khalf - 1) // khalf):
                    xt = xpool.tile([P, khalf * P], mybir.dt.float32r,
                                    name=f"xt{h}", tag="xt")
                    nc.sync.dma_start(
                        xt[:], xr[t][:, h * khalf * P:(h + 1) * khalf * P])
                    xh.append(xt)
                pts = []
                for g in range(ngrp):
                    pt = ps.tile([P, g_width], mybir.dt.float32,
                                 name=f"acc{g}", tag=f"acc{g}")
                    pts.append(pt)
                for ks in range(ksub):
                    h, ksl = divmod(ks, khalf)
                    lhsT = xh[h][:, ksl * P:(ksl + 1) * P]
                    for g in range(ngrp):
                        nc.tensor.matmul(pts[g][:], lhsT,
                                         qw[:, ks, g * g_width:(g + 1) * g_width],
                                         start=(ks == 0), stop=(ks == ksub - 1))
                for g in range(ngrp):
                    yt = ypool.tile([P, g_width], mybir.dt.float32, name="yt", tag="yt")
                    nc.scalar.copy(yt[:], pts[g][:])
                    nc.sync.dma_start(
                        y.ap()[t * P:(t + 1) * P, g * g_width:(g + 1) * g_width],
                        yt[:])
    nc.finalize()
    return nc


def _prep_x(xs):
    """[tok, kin] f32 -> tile-linearized [tok, kin] where row t*P+p holds
    x.T[128ks+p, 128t+col] at col ks*P+col (SBUF DMA order)."""
    tok, kin = xs.shape
    nt, ks = tok // P, kin // P
    # want out[t, p, ks, col] = xs[t*P+col, ks*P+p]
    return np.ascontiguousarray(
        xs.reshape(nt, P, ks, P).transpose(0, 3, 2, 1).reshape(tok, kin))


_NC_CACHE = {}


def prep_in_maps(x, weight, alpha, write_count):
    x = np.asarray(x)
    weight = np.asarray(weight)
    alpha = np.asarray(alpha)
    write_count = np.asarray(write_count)
    a11 = alpha.reshape(1, 1).astype(np.float32)
    in_maps = []
    xl = {}
    for b in range(BATCH_WAYS):
        xs = x[b * (B // BATCH_WAYS):(b + 1) * (B // BATCH_WAYS)].reshape(TOK, IN_F)
        xl[b] = _prep_x(np.ascontiguousarray(xs))
    for c in range(N_CORES):
        b, q = divmod(c, OUT_WAYS)
        wT = np.ascontiguousarray(weight[q * O:(q + 1) * O, :].T)       # [IN_F, O]
        wcT = np.ascontiguousarray(
            write_count[q * O:(q + 1) * O, :].T).astype(np.int32)
        in_maps.append({"xlin": xl[b], "wt": wT, "wct": wcT, "alpha": a11})
    return in_maps


def assemble(results):
    """results: list of 8 per-core dicts with 'y' [TOK, O]."""
    y = np.empty((B * S, OUT_F), dtype=np.float32)
    for c in range(N_CORES):
        b, q = divmod(c, OUT_WAYS)
        y[b * TOK:(b + 1) * TOK, q * O:(q + 1) * O] = results[c]["y"]
    return y.reshape(B, S, OUT_F)


def kernel(x, weight, alpha, write_count):
    if "full" not in _NC_CACHE:
        _NC_CACHE["full"] = build()
    nc = _NC_CACHE["full"]
    in_maps = prep_in_maps(x, weight, alpha, write_count)
    res = run_bass_kernel_spmd(nc, in_maps, core_ids=list(range(N_CORES)))
    return assemble(res.results)
